# revision 3
# baseline (speedup 1.0000x reference)
"""DinkNet GCN encoder kernel for one TRN2 chip (8 NeuronCores), Bass/Tile.

Math (reference):
    h   = x @ W                     (512 -> 128)
    z1  = PReLU(segsum(h[src]*no[src]) * ni + b)        # clean encoder
    z2  = same with x[perm]                             # corrupted encoder
    out = concat((z1 @ mlp_W + mlp_b).sum(1), (z2 @ ...).sum(1))

Key transformations used here:
    * x[perm] @ W == (x @ W)[perm]  -> host folds perm into the projection input
    * (z @ mlp_W + mlp_b).sum(1) == PReLU(y) . mlp_W.sum(1) + mlp_b.sum()
    * norm_out is folded into the projected rows (per-row scale)
    * segment_sum becomes one-hot matmuls accumulating in PSUM over
      dst-sorted edge tiles; edge rows are fetched with dma_gather from the
      all-gathered hcat table (hcat[i] = [h[i]*no[i] | h[perm[i]]*no[i]], bf16)

Sharding: nodes (and their output rows) are split contiguously across the 8
cores; each core owns the edges whose dst lands in its shard.  The projection
is sharded by node and the 6.4MB/core hcat shard is AllGathered.
"""
import sys

sys.path.insert(0, "/opt/trn_rl_repo")

import numpy as np
import ml_dtypes

from concourse import bass, bacc, mybir, tile, bass_utils

N = 100000
E = 1600000
NIN = 512
NH = 128
NC = 8
SHARD = N // NC                 # 12500
NB = (SHARD + 127) // 128       # 98 dst blocks per core
PAD = NB * 128                  # 12544 padded shard rows
D = 2 * NH                      # 256: [clean | corrupted]
CHUNK = 32768                   # int16 index range per dma_gather source slice
NCH = (N + CHUNK - 1) // CHUNK  # 4
CH_STARTS = [i * CHUNK for i in range(NCH)]
CH_SIZES = [min(CHUNK, N - s) for s in CH_STARTS]
BG = 4                          # dst blocks per PSUM group
RGRP = 1024                     # projection row-group width (xT columns)

BF16 = ml_dtypes.bfloat16
F32 = mybir.dt.float32
BF = mybir.dt.bfloat16
I16 = mybir.dt.int16

LAST = {}
_CACHE = {}


# --------------------------------------------------------------------------
# host preprocessing
# --------------------------------------------------------------------------
def _prep(x, src, dst, perm, W, b, alpha, mlp_W, mlp_b):
    x = np.asarray(x, np.float32)
    src = np.asarray(src, np.int64)
    dst = np.asarray(dst, np.int64)
    perm = np.asarray(perm, np.int64)
    W = np.asarray(W, np.float32)
    b = np.asarray(b, np.float32)
    alpha = np.asarray(alpha, np.float32)
    mlp_W = np.asarray(mlp_W, np.float32)
    mlp_b = np.asarray(mlp_b, np.float32)

    norm_out = np.clip(np.bincount(src, minlength=N), 1.0, None) ** -0.5
    norm_in = np.clip(np.bincount(dst, minlength=N), 1.0, None) ** -0.5
    norm_out = norm_out.astype(np.float32)
    norm_in = norm_in.astype(np.float32)

    core = dst // SHARD
    blk = (dst - core * SHARD) // 128
    chunk = src // CHUNK
    key = (core * NB + blk) * NCH + chunk
    order = np.argsort(key, kind="stable")
    src_s = src[order]
    dst_s = dst[order]

    counts = np.bincount(key, minlength=NC * NB * NCH).reshape(NC, NB, NCH)
    Tbk = (counts.max(axis=0) + 127) // 128
    Tbk[Tbk.sum(axis=1) == 0, 0] = 1

    ngroups = (NB + BG - 1) // BG
    tile_off = {}
    stream = []                        # (block, chunk, ntiles) in stream order
    t = 0
    for g in range(ngroups):
        for k in range(NCH):
            for bb in range(g * BG, min((g + 1) * BG, NB)):
                nt = int(Tbk[bb, k])
                if nt == 0:
                    continue
                tile_off[(bb, k)] = t
                stream.append((bb, k, nt))
                t += nt
    T_total = t
    SLOTS = T_total * 128

    cum = np.zeros(NC * NB * NCH + 1, np.int64)
    np.cumsum(np.bincount(key, minlength=NC * NB * NCH), out=cum[1:])

    xp = x[perm]
    u = mlp_W.sum(axis=1).astype(np.float32)
    b2 = np.tile(np.concatenate([b, b])[None, :], (128, 1)).astype(np.float32)
    u2 = np.tile(np.concatenate([u, u])[None, :], (128, 1)).astype(np.float32)
    au2 = np.tile(
        np.concatenate([alpha * u, alpha * u])[None, :], (128, 1)
    ).astype(np.float32)
    consts = np.ascontiguousarray(np.concatenate([b2, u2, au2], axis=1))
    Wb = np.ascontiguousarray(W.astype(BF16))
    bsum = float(mlp_b.sum())

    in_maps = []
    for c in range(NC):
        srcloc = np.zeros(SLOTS, np.int64)
        dstloc = np.full(SLOTS, -1.0, np.float32)
        for (bb, k, nt) in stream:
            kk = (c * NB + bb) * NCH + k
            e0, e1 = cum[kk], cum[kk + 1]
            s0 = tile_off[(bb, k)] * 128
            srcloc[s0 : s0 + (e1 - e0)] = src_s[e0:e1] - CH_STARTS[k]
            dstloc[s0 : s0 + (e1 - e0)] = (
                dst_s[e0:e1] - c * SHARD - bb * 128
            ).astype(np.float32)
        wrap = np.ascontiguousarray(srcloc.astype(np.int16).reshape(-1, 16).T)
        idx16 = np.ascontiguousarray(np.tile(wrap, (8, 1)))
        dst_slab = np.ascontiguousarray(dstloc.reshape(T_total, 128).T)

        base = c * SHARD
        xs = np.zeros((PAD, NIN), np.float32)
        xs[:SHARD] = x[base : base + SHARD]
        xps = np.zeros((PAD, NIN), np.float32)
        xps[:SHARD] = xp[base : base + SHARD]
        no = np.zeros(PAD, np.float32)
        no[:SHARD] = norm_out[base : base + SHARD]
        ni = np.zeros(PAD, np.float32)
        ni[:SHARD] = norm_in[base : base + SHARD]

        in_maps.append(
            dict(
                xT=np.ascontiguousarray(xs.T).astype(BF16),
                xpT=np.ascontiguousarray(xps.T).astype(BF16),
                Wb=Wb,
                consts=consts,
                no_slab=np.ascontiguousarray(no.reshape(NB, 128).T),
                ni_slab=np.ascontiguousarray(ni.reshape(NB, 128).T),
                dst_slab=dst_slab,
                idx16=idx16,
            )
        )
    meta = dict(
        Tbk=Tbk, stream=stream, tile_off=tile_off, T_total=T_total, bsum=bsum,
        ngroups=ngroups,
    )
    return in_maps, meta


# --------------------------------------------------------------------------
# device program
# --------------------------------------------------------------------------
def _build(meta):
    Tbk = meta["Tbk"]
    stream = meta["stream"]
    tile_off = meta["tile_off"]
    T_total = meta["T_total"]
    bsum = meta["bsum"]
    ngroups = meta["ngroups"]

    # per-block first/last tile in stream order (for PSUM start/stop flags)
    first_tile = {}
    last_tile = {}
    for (bb, k, nt) in stream:
        t0 = tile_off[(bb, k)]
        if bb not in first_tile:
            first_tile[bb] = t0
        last_tile[bb] = t0 + nt - 1

    nc = bacc.Bacc(
        "TRN2", target_bir_lowering=False, debug=False, num_devices=NC
    )
    xT_d = nc.dram_tensor("xT", [NIN, PAD], BF, kind="ExternalInput")
    xpT_d = nc.dram_tensor("xpT", [NIN, PAD], BF, kind="ExternalInput")
    Wb_d = nc.dram_tensor("Wb", [NIN, NH], BF, kind="ExternalInput")
    consts_d = nc.dram_tensor("consts", [128, 768], F32, kind="ExternalInput")
    no_d = nc.dram_tensor("no_slab", [128, NB], F32, kind="ExternalInput")
    ni_d = nc.dram_tensor("ni_slab", [128, NB], F32, kind="ExternalInput")
    dst_d = nc.dram_tensor("dst_slab", [128, T_total], F32, kind="ExternalInput")
    idx_d = nc.dram_tensor("idx16", [128, T_total * 8], I16, kind="ExternalInput")
    out_d = nc.dram_tensor("out_raw", [128, 2 * NB], F32, kind="ExternalOutput")

    AL = mybir.AluOpType

    with tile.TileContext(nc) as tc:
        with tc.tile_pool(name="dram", bufs=1, space="DRAM") as dramp:
            hcat_in = dramp.tile([SHARD, D], BF)
            hcat_full = dramp.tile([N, D], BF, addr_space="Shared")

            with tc.tile_pool(name="cst", bufs=1) as cp:
                Wk_t = cp.tile([128, NIN], BF)           # 4 chunks of W side by side
                for k in range(4):
                    nc.sync.dma_start(
                        out=Wk_t[:, k * NH : (k + 1) * NH],
                        in_=Wb_d[k * 128 : (k + 1) * 128, :],
                    )
                cst_t = cp.tile([128, 768], F32)
                nc.sync.dma_start(out=cst_t[:], in_=consts_d[:])
                b2 = cst_t[:, 0:256]
                u2 = cst_t[:, 256:512]
                au2 = cst_t[:, 512:768]
                no_sb = cp.tile([128, NB], F32)
                nc.sync.dma_start(out=no_sb[:], in_=no_d[:])
                ni_sb = cp.tile([128, NB], F32)
                nc.sync.dma_start(out=ni_sb[:], in_=ni_d[:])
                dst_sb = cp.tile([128, T_total], F32)
                nc.sync.dma_start(out=dst_sb[:], in_=dst_d[:])
                iota_t = cp.tile([128, 128], F32)
                nc.gpsimd.iota(
                    iota_t[:], pattern=[[1, 128]], base=0, channel_multiplier=0,
                    allow_small_or_imprecise_dtypes=True,
                )

                # ---------------- phase A: projection ----------------
                with (
                    tc.tile_pool(name="xk", bufs=2) as xkp,
                    tc.tile_pool(name="hc", bufs=4) as hcp,
                    tc.tile_pool(name="pps", bufs=4, space="PSUM") as ppsp,
                ):
                    col0 = 0
                    while col0 < PAD:
                        cols = min(RGRP, PAD - col0)
                        xk_t, xpk_t = [], []
                        for k in range(4):
                            xt = xkp.tile([128, cols], BF, tag=f"xk{k}")
                            nc.sync.dma_start(
                                out=xt[:],
                                in_=xT_d[k * 128 : (k + 1) * 128, col0 : col0 + cols],
                            )
                            xk_t.append(xt)
                        for k in range(4):
                            xt = xkp.tile([128, cols], BF, tag=f"xpk{k}")
                            nc.sync.dma_start(
                                out=xt[:],
                                in_=xpT_d[k * 128 : (k + 1) * 128, col0 : col0 + cols],
                            )
                            xpk_t.append(xt)
                        for rt in range(cols // 128):
                            B = (col0 + rt * 128) // 128
                            ps = ppsp.tile([128, D], F32, tag="pps")
                            for k in range(4):
                                nc.tensor.matmul(
                                    out=ps[:, :NH],
                                    lhsT=xk_t[k][:, rt * 128 : (rt + 1) * 128],
                                    rhs=Wk_t[:, k * NH : (k + 1) * NH],
                                    start=(k == 0),
                                    stop=(k == 3),
                                )
                            for k in range(4):
                                nc.tensor.matmul(
                                    out=ps[:, NH:],
                                    lhsT=xpk_t[k][:, rt * 128 : (rt + 1) * 128],
                                    rhs=Wk_t[:, k * NH : (k + 1) * NH],
                                    start=(k == 0),
                                    stop=(k == 3),
                                )
                            hc = hcp.tile([128, D], BF, tag="hc")
                            nc.vector.tensor_scalar(
                                out=hc[:], in0=ps[:],
                                scalar1=no_sb[:, B : B + 1], scalar2=None,
                                op0=AL.mult,
                            )
                            rows = min(128, SHARD - B * 128)
                            nc.sync.dma_start(
                                out=hcat_in[B * 128 : B * 128 + rows, :],
                                in_=hc[:rows, :],
                            )
                        col0 += cols

                # ---------------- phase B: all-gather ----------------
                nc.gpsimd.collective_compute(
                    "AllGather",
                    mybir.AluOpType.bypass,
                    replica_groups=[list(range(NC))],
                    ins=[hcat_in.opt()],
                    outs=[hcat_full.opt()],
                )

                # ---------------- phase C: edge aggregation ----------------
                with (
                    tc.tile_pool(name="idx", bufs=3) as idxp,
                    tc.tile_pool(name="gb", bufs=3) as gbp,
                    tc.tile_pool(name="oh", bufs=8) as ohp,
                    tc.tile_pool(name="post", bufs=4) as postp,
                    tc.tile_pool(name="acc", bufs=8) as accp,
                    tc.tile_pool(name="aps", bufs=8, space="PSUM") as apsp,
                    tc.tile_pool(name="outs", bufs=1) as outp,
                ):
                    out_stage = outp.tile([128, 2 * NB], F32)
                    for g in range(ngroups):
                        blocks = list(range(g * BG, min((g + 1) * BG, NB)))
                        gbufs = {}
                        for k in range(NCH):
                            seg = [
                                (bb, k, int(Tbk[bb, k]))
                                for bb in blocks
                                if Tbk[bb, k] > 0
                            ]
                            if not seg:
                                continue
                            Tgk = sum(nt for _, _, nt in seg)
                            t0 = tile_off[(seg[0][0], k)]
                            idxt = idxp.tile([128, Tgk * 8], I16, tag="idx")
                            nc.sync.dma_start(
                                out=idxt[:],
                                in_=idx_d[:, t0 * 8 : (t0 + Tgk) * 8],
                            )
                            gb = gbp.tile([128, Tgk * D], BF, tag="gb")
                            nc.gpsimd.dma_gather(
                                out_ap=gb[:].rearrange("p (t d) -> p t d", d=D),
                                in_ap=hcat_full[
                                    CH_STARTS[k] : CH_STARTS[k] + CH_SIZES[k], :
                                ],
                                idxs_ap=idxt[:],
                                num_idxs=Tgk * 128,
                                num_idxs_reg=Tgk * 128,
                                elem_size=D,
                                single_packet=False,
                            )
                            gbufs[k] = (gb, t0)
                        psums = {}
                        for bb in blocks:
                            psums[bb] = apsp.tile([128, D], F32, tag="aps", name=f"aps{bb}")
                        for k in range(NCH):
                            if k not in gbufs:
                                continue
                            gb, t0k = gbufs[k]
                            for bb in blocks:
                                nt = int(Tbk[bb, k])
                                if nt == 0:
                                    continue
                                tstart = tile_off[(bb, k)]
                                for tl in range(nt):
                                    t = tstart + tl
                                    oh = ohp.tile([128, 128], BF, tag="oh")
                                    nc.vector.tensor_scalar(
                                        out=oh[:], in0=iota_t[:],
                                        scalar1=dst_sb[:, t : t + 1], scalar2=None,
                                        op0=AL.is_equal,
                                    )
                                    c0 = (t - t0k) * D
                                    nc.tensor.matmul(
                                        out=psums[bb][:],
                                        lhsT=oh[:],
                                        rhs=gb[:, c0 : c0 + D],
                                        start=(t == first_tile[bb]),
                                        stop=(t == last_tile[bb]),
                                    )
                        for bb in blocks:
                            ps = psums[bb]
                            y = postp.tile([128, D], F32, tag="y")
                            nc.vector.scalar_tensor_tensor(
                                out=y[:], in0=ps[:],
                                scalar=ni_sb[:, bb : bb + 1], in1=b2,
                                op0=AL.mult, op1=AL.add,
                            )
                            scr = postp.tile([128, 128], F32, tag="scr")
                            a = accp.tile([128, 4], F32, tag="a")
                            for half in range(2):
                                hs = slice(half * NH, (half + 1) * NH)
                                nc.vector.scalar_tensor_tensor(
                                    out=scr[:], in0=y[:, hs], scalar=0.0,
                                    in1=u2[:, hs], op0=AL.max, op1=AL.mult,
                                    accum_out=a[:, 2 * half : 2 * half + 1],
                                )
                                nc.vector.scalar_tensor_tensor(
                                    out=scr[:], in0=y[:, hs], scalar=0.0,
                                    in1=au2[:, hs], op0=AL.min, op1=AL.mult,
                                    accum_out=a[:, 2 * half + 1 : 2 * half + 2],
                                )
                            nc.vector.tensor_add(
                                out=out_stage[:, 2 * bb : 2 * bb + 1],
                                in0=a[:, 0:1], in1=a[:, 1:2],
                            )
                            nc.vector.tensor_add(
                                out=out_stage[:, 2 * bb + 1 : 2 * bb + 2],
                                in0=a[:, 2:3], in1=a[:, 3:4],
                            )
                    nc.vector.tensor_scalar(
                        out=out_stage[:], in0=out_stage[:],
                        scalar1=bsum, scalar2=None, op0=AL.add,
                    )
                    nc.sync.dma_start(out=out_d[:], in_=out_stage[:])

    nc.compile()
    return nc


# --------------------------------------------------------------------------
# entry point
# --------------------------------------------------------------------------
def kernel(x, src, dst, perm, W, b, alpha, mlp_W, mlp_b, batch_train=0, **_):
    in_maps, meta = _prep(x, src, dst, perm, W, b, alpha, mlp_W, mlp_b)

    sig = meta["Tbk"].tobytes()
    if sig in _CACHE:
        nc = _CACHE[sig]
    else:
        nc = _build(meta)
        _CACHE.clear()
        _CACHE[sig] = nc

    res = bass_utils.run_bass_kernel_spmd(
        nc, in_maps, core_ids=list(range(NC))
    )
    LAST["exec_time_ns"] = res.exec_time_ns
    LAST["results"] = None

    out1 = np.zeros(N, np.float32)
    out2 = np.zeros(N, np.float32)
    for c in range(NC):
        o = np.asarray(res.results[c]["out_raw"], np.float32)
        out1[c * SHARD : (c + 1) * SHARD] = o[:, 0::2].T.reshape(-1)[:SHARD]
        out2[c * SHARD : (c + 1) * SHARD] = o[:, 1::2].T.reshape(-1)[:SHARD]
    return np.concatenate([out1, out2])


# revision 4
# speedup vs baseline: 1.0575x; 1.0575x over previous
"""DinkNet GCN encoder kernel for one TRN2 chip (8 NeuronCores), Bass/Tile.

Math (reference):
    h   = x @ W                     (512 -> 128)
    z1  = PReLU(segsum(h[src]*no[src]) * ni + b)        # clean encoder
    z2  = same with x[perm]                             # corrupted encoder
    out = concat((z1 @ mlp_W + mlp_b).sum(1), (z2 @ ...).sum(1))

Key transformations used here:
    * x[perm] @ W == (x @ W)[perm]  -> host folds perm into the projection input
    * (z @ mlp_W + mlp_b).sum(1) == PReLU(y) . mlp_W.sum(1) + mlp_b.sum()
    * norm_out is folded into the projected rows (per-row scale)
    * segment_sum becomes one-hot matmuls accumulating in PSUM over
      dst-sorted edge tiles; edge rows are fetched with dma_gather from the
      all-gathered hcat table (hcat[i] = [h[i]*no[i] | h[perm[i]]*no[i]], bf16)

Sharding: nodes (and their output rows) are split contiguously across the 8
cores; each core owns the edges whose dst lands in its shard.  The projection
is sharded by node and the 6.4MB/core hcat shard is AllGathered.
"""
import sys

sys.path.insert(0, "/opt/trn_rl_repo")

import numpy as np
import ml_dtypes

from concourse import bass, bacc, mybir, tile, bass_utils

N = 100000
E = 1600000
NIN = 512
NH = 128
NC = 8
SHARD = N // NC                 # 12500
NB = (SHARD + 127) // 128       # 98 dst blocks per core
PAD = NB * 128                  # 12544 padded shard rows
D = 2 * NH                      # 256: [clean | corrupted]
CHUNK = 32768                   # int16 index range per dma_gather source slice
NCH = (N + CHUNK - 1) // CHUNK  # 4
CH_STARTS = [i * CHUNK for i in range(NCH)]
CH_SIZES = [min(CHUNK, N - s) for s in CH_STARTS]
BG = 4                          # dst blocks per PSUM group
RGRP = 1024                     # projection row-group width (xT columns)

BF16 = ml_dtypes.bfloat16
F32 = mybir.dt.float32
BF = mybir.dt.bfloat16
I16 = mybir.dt.int16

LAST = {}
_CACHE = {}


# --------------------------------------------------------------------------
# host preprocessing
# --------------------------------------------------------------------------
def _prep(x, src, dst, perm, W, b, alpha, mlp_W, mlp_b):
    x = np.asarray(x, np.float32)
    src = np.asarray(src, np.int64)
    dst = np.asarray(dst, np.int64)
    perm = np.asarray(perm, np.int64)
    W = np.asarray(W, np.float32)
    b = np.asarray(b, np.float32)
    alpha = np.asarray(alpha, np.float32)
    mlp_W = np.asarray(mlp_W, np.float32)
    mlp_b = np.asarray(mlp_b, np.float32)

    norm_out = np.clip(np.bincount(src, minlength=N), 1.0, None) ** -0.5
    norm_in = np.clip(np.bincount(dst, minlength=N), 1.0, None) ** -0.5
    norm_out = norm_out.astype(np.float32)
    norm_in = norm_in.astype(np.float32)

    core = dst // SHARD
    blk = (dst - core * SHARD) // 128
    chunk = src // CHUNK
    key = (core * NB + blk) * NCH + chunk
    order = np.argsort(key, kind="stable")
    src_s = src[order]
    dst_s = dst[order]

    counts = np.bincount(key, minlength=NC * NB * NCH).reshape(NC, NB, NCH)
    Tbk = (counts.max(axis=0) + 127) // 128
    Tbk[Tbk.sum(axis=1) == 0, 0] = 1

    ngroups = (NB + BG - 1) // BG
    tile_off = {}
    stream = []                        # (block, chunk, ntiles) in stream order
    t = 0
    for g in range(ngroups):
        for k in range(NCH):
            for bb in range(g * BG, min((g + 1) * BG, NB)):
                nt = int(Tbk[bb, k])
                if nt == 0:
                    continue
                tile_off[(bb, k)] = t
                stream.append((bb, k, nt))
                t += nt
    T_total = t
    SLOTS = T_total * 128

    cum = np.zeros(NC * NB * NCH + 1, np.int64)
    np.cumsum(np.bincount(key, minlength=NC * NB * NCH), out=cum[1:])

    xp = x[perm]
    u = mlp_W.sum(axis=1).astype(np.float32)
    b2 = np.tile(np.concatenate([b, b])[None, :], (128, 1)).astype(np.float32)
    u2 = np.tile(np.concatenate([u, u])[None, :], (128, 1)).astype(np.float32)
    au2 = np.tile(
        np.concatenate([alpha * u, alpha * u])[None, :], (128, 1)
    ).astype(np.float32)
    consts = np.ascontiguousarray(np.concatenate([b2, u2, au2], axis=1))
    Wb = np.ascontiguousarray(W.astype(BF16))
    bsum = float(mlp_b.sum())

    in_maps = []
    for c in range(NC):
        srcloc = np.zeros(SLOTS, np.int64)
        dstloc = np.full(SLOTS, -1.0, np.float32)
        for (bb, k, nt) in stream:
            kk = (c * NB + bb) * NCH + k
            e0, e1 = cum[kk], cum[kk + 1]
            s0 = tile_off[(bb, k)] * 128
            srcloc[s0 : s0 + (e1 - e0)] = src_s[e0:e1] - CH_STARTS[k]
            dstloc[s0 : s0 + (e1 - e0)] = (
                dst_s[e0:e1] - c * SHARD - bb * 128
            ).astype(np.float32)
        wrap = np.ascontiguousarray(srcloc.astype(np.int16).reshape(-1, 16).T)
        idx16 = np.ascontiguousarray(np.tile(wrap, (8, 1)))
        dst_slab = np.ascontiguousarray(dstloc.reshape(T_total, 128).T)

        base = c * SHARD
        xs = np.zeros((PAD, NIN), np.float32)
        xs[:SHARD] = x[base : base + SHARD]
        xps = np.zeros((PAD, NIN), np.float32)
        xps[:SHARD] = xp[base : base + SHARD]
        no = np.zeros(PAD, np.float32)
        no[:SHARD] = norm_out[base : base + SHARD]
        ni = np.zeros(PAD, np.float32)
        ni[:SHARD] = norm_in[base : base + SHARD]

        in_maps.append(
            dict(
                xT=np.ascontiguousarray(xs.T).astype(BF16),
                xpT=np.ascontiguousarray(xps.T).astype(BF16),
                Wb=Wb,
                consts=consts,
                no_slab=np.ascontiguousarray(no.reshape(NB, 128).T),
                ni_slab=np.ascontiguousarray(ni.reshape(NB, 128).T),
                dst_slab=dst_slab,
                idx16=idx16,
            )
        )
    meta = dict(
        Tbk=Tbk, stream=stream, tile_off=tile_off, T_total=T_total, bsum=bsum,
        ngroups=ngroups,
    )
    return in_maps, meta


# --------------------------------------------------------------------------
# device program
# --------------------------------------------------------------------------
def _build(meta):
    Tbk = meta["Tbk"]
    stream = meta["stream"]
    tile_off = meta["tile_off"]
    T_total = meta["T_total"]
    bsum = meta["bsum"]
    ngroups = meta["ngroups"]

    # per-block first/last tile in stream order (for PSUM start/stop flags)
    first_tile = {}
    last_tile = {}
    for (bb, k, nt) in stream:
        t0 = tile_off[(bb, k)]
        if bb not in first_tile:
            first_tile[bb] = t0
        last_tile[bb] = t0 + nt - 1

    nc = bacc.Bacc(
        "TRN2", target_bir_lowering=False, debug=False, num_devices=NC,
        num_swdge_queues=4,
    )
    xT_d = nc.dram_tensor("xT", [NIN, PAD], BF, kind="ExternalInput")
    xpT_d = nc.dram_tensor("xpT", [NIN, PAD], BF, kind="ExternalInput")
    Wb_d = nc.dram_tensor("Wb", [NIN, NH], BF, kind="ExternalInput")
    consts_d = nc.dram_tensor("consts", [128, 768], F32, kind="ExternalInput")
    no_d = nc.dram_tensor("no_slab", [128, NB], F32, kind="ExternalInput")
    ni_d = nc.dram_tensor("ni_slab", [128, NB], F32, kind="ExternalInput")
    dst_d = nc.dram_tensor("dst_slab", [128, T_total], F32, kind="ExternalInput")
    idx_d = nc.dram_tensor("idx16", [128, T_total * 8], I16, kind="ExternalInput")
    out_d = nc.dram_tensor("out_raw", [128, 2 * NB], F32, kind="ExternalOutput")

    AL = mybir.AluOpType

    with tile.TileContext(nc) as tc:
        with tc.tile_pool(name="dram", bufs=1, space="DRAM") as dramp:
            hcat_in = dramp.tile([SHARD, D], BF)
            hcat_full = dramp.tile([N, D], BF, addr_space="Shared")

            with tc.tile_pool(name="cst", bufs=1) as cp:
                Wk_t = cp.tile([128, NIN], BF)           # 4 chunks of W side by side
                for k in range(4):
                    nc.sync.dma_start(
                        out=Wk_t[:, k * NH : (k + 1) * NH],
                        in_=Wb_d[k * 128 : (k + 1) * 128, :],
                    )
                cst_t = cp.tile([128, 768], F32)
                nc.sync.dma_start(out=cst_t[:], in_=consts_d[:])
                b2 = cst_t[:, 0:256]
                u2 = cst_t[:, 256:512]
                au2 = cst_t[:, 512:768]
                no_sb = cp.tile([128, NB], F32)
                nc.sync.dma_start(out=no_sb[:], in_=no_d[:])
                ni_sb = cp.tile([128, NB], F32)
                nc.sync.dma_start(out=ni_sb[:], in_=ni_d[:])
                dst_sb = cp.tile([128, T_total], F32)
                nc.sync.dma_start(out=dst_sb[:], in_=dst_d[:])
                iota_t = cp.tile([128, 128], F32)
                nc.gpsimd.iota(
                    iota_t[:], pattern=[[1, 128]], base=0, channel_multiplier=0,
                    allow_small_or_imprecise_dtypes=True,
                )

                # ---------------- phase A: projection ----------------
                with (
                    tc.tile_pool(name="xk", bufs=2) as xkp,
                    tc.tile_pool(name="hc", bufs=4) as hcp,
                    tc.tile_pool(name="pps", bufs=4, space="PSUM") as ppsp,
                ):
                    col0 = 0
                    while col0 < PAD:
                        cols = min(RGRP, PAD - col0)
                        xk_t, xpk_t = [], []
                        for k in range(4):
                            xt = xkp.tile([128, cols], BF, tag=f"xk{k}")
                            nc.sync.dma_start(
                                out=xt[:],
                                in_=xT_d[k * 128 : (k + 1) * 128, col0 : col0 + cols],
                            )
                            xk_t.append(xt)
                        for k in range(4):
                            xt = xkp.tile([128, cols], BF, tag=f"xpk{k}")
                            nc.sync.dma_start(
                                out=xt[:],
                                in_=xpT_d[k * 128 : (k + 1) * 128, col0 : col0 + cols],
                            )
                            xpk_t.append(xt)
                        for rt in range(cols // 128):
                            B = (col0 + rt * 128) // 128
                            ps = ppsp.tile([128, D], F32, tag="pps")
                            for k in range(4):
                                nc.tensor.matmul(
                                    out=ps[:, :NH],
                                    lhsT=xk_t[k][:, rt * 128 : (rt + 1) * 128],
                                    rhs=Wk_t[:, k * NH : (k + 1) * NH],
                                    start=(k == 0),
                                    stop=(k == 3),
                                )
                            for k in range(4):
                                nc.tensor.matmul(
                                    out=ps[:, NH:],
                                    lhsT=xpk_t[k][:, rt * 128 : (rt + 1) * 128],
                                    rhs=Wk_t[:, k * NH : (k + 1) * NH],
                                    start=(k == 0),
                                    stop=(k == 3),
                                )
                            hc = hcp.tile([128, D], BF, tag="hc")
                            nc.vector.tensor_scalar(
                                out=hc[:], in0=ps[:],
                                scalar1=no_sb[:, B : B + 1], scalar2=None,
                                op0=AL.mult,
                            )
                            rows = min(128, SHARD - B * 128)
                            nc.sync.dma_start(
                                out=hcat_in[B * 128 : B * 128 + rows, :],
                                in_=hc[:rows, :],
                            )
                        col0 += cols

                # ---------------- phase B: all-gather ----------------
                nc.gpsimd.collective_compute(
                    "AllGather",
                    mybir.AluOpType.bypass,
                    replica_groups=[list(range(NC))],
                    ins=[hcat_in.opt()],
                    outs=[hcat_full.opt()],
                )

                # ---------------- phase C: edge aggregation ----------------
                with (
                    tc.tile_pool(name="idx", bufs=3) as idxp,
                    tc.tile_pool(name="gb", bufs=4) as gbp,
                    tc.tile_pool(name="oh", bufs=8) as ohp,
                    tc.tile_pool(name="post", bufs=4) as postp,
                    tc.tile_pool(name="acc", bufs=8) as accp,
                    tc.tile_pool(name="aps", bufs=8, space="PSUM") as apsp,
                    tc.tile_pool(name="outs", bufs=1) as outp,
                ):
                    out_stage = outp.tile([128, 2 * NB], F32)
                    for g in range(ngroups):
                        blocks = list(range(g * BG, min((g + 1) * BG, NB)))
                        gbufs = {}
                        for k in range(NCH):
                            seg = [
                                (bb, k, int(Tbk[bb, k]))
                                for bb in blocks
                                if Tbk[bb, k] > 0
                            ]
                            if not seg:
                                continue
                            Tgk = sum(nt for _, _, nt in seg)
                            t0 = tile_off[(seg[0][0], k)]
                            idxt = idxp.tile([128, Tgk * 8], I16, tag="idx")
                            nc.sync.dma_start(
                                out=idxt[:],
                                in_=idx_d[:, t0 * 8 : (t0 + Tgk) * 8],
                            )
                            gb = gbp.tile([128, Tgk * D], BF, tag="gb")
                            nc.gpsimd.dma_gather(
                                out_ap=gb[:].rearrange("p (t d) -> p t d", d=D),
                                in_ap=hcat_full[
                                    CH_STARTS[k] : CH_STARTS[k] + CH_SIZES[k], :
                                ],
                                idxs_ap=idxt[:],
                                num_idxs=Tgk * 128,
                                num_idxs_reg=Tgk * 128,
                                elem_size=D,
                                single_packet=False,
                                queue_num=k,
                            )
                            gbufs[k] = (gb, t0)
                        psums = {}
                        for bb in blocks:
                            psums[bb] = apsp.tile([128, D], F32, tag="aps", name=f"aps{bb}")
                        for k in range(NCH):
                            if k not in gbufs:
                                continue
                            gb, t0k = gbufs[k]
                            for bb in blocks:
                                nt = int(Tbk[bb, k])
                                if nt == 0:
                                    continue
                                tstart = tile_off[(bb, k)]
                                for tl in range(nt):
                                    t = tstart + tl
                                    oh = ohp.tile([128, 128], BF, tag="oh")
                                    nc.vector.tensor_scalar(
                                        out=oh[:], in0=iota_t[:],
                                        scalar1=dst_sb[:, t : t + 1], scalar2=None,
                                        op0=AL.is_equal,
                                    )
                                    c0 = (t - t0k) * D
                                    nc.tensor.matmul(
                                        out=psums[bb][:],
                                        lhsT=oh[:],
                                        rhs=gb[:, c0 : c0 + D],
                                        start=(t == first_tile[bb]),
                                        stop=(t == last_tile[bb]),
                                    )
                        for bb in blocks:
                            ps = psums[bb]
                            y = postp.tile([128, D], F32, tag="y")
                            nc.vector.scalar_tensor_tensor(
                                out=y[:], in0=ps[:],
                                scalar=ni_sb[:, bb : bb + 1], in1=b2,
                                op0=AL.mult, op1=AL.add,
                            )
                            scr = postp.tile([128, 128], F32, tag="scr")
                            a = accp.tile([128, 4], F32, tag="a")
                            for half in range(2):
                                hs = slice(half * NH, (half + 1) * NH)
                                nc.vector.scalar_tensor_tensor(
                                    out=scr[:], in0=y[:, hs], scalar=0.0,
                                    in1=u2[:, hs], op0=AL.max, op1=AL.mult,
                                    accum_out=a[:, 2 * half : 2 * half + 1],
                                )
                                nc.vector.scalar_tensor_tensor(
                                    out=scr[:], in0=y[:, hs], scalar=0.0,
                                    in1=au2[:, hs], op0=AL.min, op1=AL.mult,
                                    accum_out=a[:, 2 * half + 1 : 2 * half + 2],
                                )
                            nc.vector.tensor_add(
                                out=out_stage[:, 2 * bb : 2 * bb + 1],
                                in0=a[:, 0:1], in1=a[:, 1:2],
                            )
                            nc.vector.tensor_add(
                                out=out_stage[:, 2 * bb + 1 : 2 * bb + 2],
                                in0=a[:, 2:3], in1=a[:, 3:4],
                            )
                    nc.vector.tensor_scalar(
                        out=out_stage[:], in0=out_stage[:],
                        scalar1=bsum, scalar2=None, op0=AL.add,
                    )
                    nc.sync.dma_start(out=out_d[:], in_=out_stage[:])

    nc.compile()
    return nc


# --------------------------------------------------------------------------
# entry point
# --------------------------------------------------------------------------
def kernel(x, src, dst, perm, W, b, alpha, mlp_W, mlp_b, batch_train=0, **_):
    in_maps, meta = _prep(x, src, dst, perm, W, b, alpha, mlp_W, mlp_b)

    sig = meta["Tbk"].tobytes()
    if sig in _CACHE:
        nc = _CACHE[sig]
    else:
        nc = _build(meta)
        _CACHE.clear()
        _CACHE[sig] = nc

    res = bass_utils.run_bass_kernel_spmd(
        nc, in_maps, core_ids=list(range(NC))
    )
    LAST["exec_time_ns"] = res.exec_time_ns
    LAST["results"] = None

    out1 = np.zeros(N, np.float32)
    out2 = np.zeros(N, np.float32)
    for c in range(NC):
        o = np.asarray(res.results[c]["out_raw"], np.float32)
        out1[c * SHARD : (c + 1) * SHARD] = o[:, 0::2].T.reshape(-1)[:SHARD]
        out2[c * SHARD : (c + 1) * SHARD] = o[:, 1::2].T.reshape(-1)[:SHARD]
    return np.concatenate([out1, out2])
